# revision 39
# baseline (speedup 1.0000x reference)
"""Trainium2 Bass kernel for nn_AdderDeconv_new_77034533421672.

Mathematical structure of the reference network:
  - Every adder_l1 layer outputs  -sum |...|  which is strictly negative at
    every position for any generic input.
  - Each adder layer (except the last) is followed by relu(), which therefore
    outputs exactly 0.0 everywhere, and bn_t turns that into the per-channel
    constant map  h[n,c,:,:] = bn*_b[c].
  - MaxUnpool scatters non-positive values into zeros; the following relu
    zeroes those too.
  So the network output equals the last adder layer applied to the constant
  map bn25_b, with zero padding:

    y[n,co,p,q] = -sum_{ci,di,dj} ( inbounds(p+di-1, q+dj-1)
                                      ? |bn25_b[ci] - w26[co,ci,di,dj]|
                                      : |w26[co,ci,di,dj]| )

  This depends only on w26 [3,32,3,3] and bn25_b [32]; it is identical for
  all n.  With a(p,di) = [0 <= p+di-1 < 128], b(q,dj) likewise, and
  wm = |w| - |b-w| (out-of-bounds minus in-bounds tap cost):

    y[co,p,q] = -sum|w26[co,:]| + sum_{di,dj} a(p,di) b(q,dj) wm[co,di,dj]

  Everything after the elementwise |.| is linear, so the device kernel is:
  a couple of DVE ops (subtract + abs-reduce), then TWO matmuls with
  constant 0/1 matrices:
    stage 1 (K=128): [ |w|-|b-w| in (di,ci)-blocks ; -sum_t|w| in ci rows ]
                     against rhs1[., p] = [ a(p,di) ; 1 ]  ->  s1 [12, 128]
    stage 2 (K=12):  s1 against a constant block-diagonal column selector
                     r12 [12, 384]  ->  the full [128, 3*128] map.

  Written in raw Bass (no Tile framework): the dependency graph is a short
  linear chain, explicit semaphores keep every instruction within the HW
  sync-wait slot limits (PE matmul has a single wait slot; raw bass uses
  standalone WAIT instructions instead), and there is no kernel-tail
  drain/barrier overhead.

  Sharding: data-parallel over batch N (hint) — all 8 cores run the identical
  tiny program; the host gathers cores 0..3 as batch elements 0..3.
"""

import numpy as np

import concourse.bass as bass
import concourse.mybir as mybir
from concourse.bass_utils import run_bass_kernel_spmd

F32 = mybir.dt.float32
F32R = mybir.dt.float32r
ALU = mybir.AluOpType
AX = mybir.AxisListType

N_CORES = 8


def make_r12() -> np.ndarray:
    """Constant stage-2 matrix: r12[co*3+dj, co'*128+q] = (co==co')*b(q,dj),
    r12[9+co, co'*128+q] = (co==co')."""
    r12 = np.zeros((12, 384), np.float32)
    for co in range(3):
        for dj in range(3):
            row = np.ones(128, np.float32)
            if dj == 0:
                row[0] = 0.0
            if dj == 2:
                row[127] = 0.0
            r12[co * 3 + dj, co * 128 : (co + 1) * 128] = row
        r12[9 + co, co * 128 : (co + 1) * 128] = 1.0
    return r12


def make_pk(w26: np.ndarray, b: np.ndarray) -> np.ndarray:
    """Host-packed staging tensor (single DMA):
    pk[0:96, 0:9]    = W96[di*32+ci, co*3+dj] = w26[co,ci,di,dj]
    pk[96:128, 9:36] = W32[ci, co*9+t]        = w26[co,ci,t]  (t = di*3+dj)
    pk[0:96, 36]     = b96[di*32+ci]          = bn25_b[ci]
    (W32 lives on partitions 96..127 so its -sum_t|w| reduction lands on the
    same partition lanes as the m128 rows it feeds — DVE is partition-locked.)
    """
    pk = np.zeros((128, 37), np.float32)
    pk[0:96, 0:9] = w26.transpose(2, 1, 0, 3).reshape(96, 9)
    pk[96:128, 9:36] = w26.transpose(1, 0, 2, 3).reshape(32, 27)
    pk[0:96, 36] = np.tile(b, 3)
    return pk


def build_program():
    nc = bass.Bass()
    lp = nc.allow_low_precision(reason="fp32r PE operands; |values| ~ 1e2, threshold 2e-2")
    lp.__enter__()
    pk = nc.dram_tensor("pk", [128, 37], F32, kind="ExternalInput")
    r12d = nc.dram_tensor("r12const", [12, 384], F32, kind="ExternalInput")
    y = nc.dram_tensor("y", [128, 384], F32, kind="ExternalOutput")

    with (
        nc.sbuf_tensor([128, 37], F32) as pkt,
        nc.sbuf_tensor([12, 384], F32R) as rc,
        nc.sbuf_tensor([96, 9], F32) as a1,
        nc.sbuf_tensor([96, 9], F32) as t96,
        nc.sbuf_tensor([96, 9], F32) as u96,
        nc.sbuf_tensor([128, 12], F32) as m128,
        nc.sbuf_tensor([128, 128], F32) as rhs1,
        nc.sbuf_tensor([12, 128], F32R) as s1,
        nc.sbuf_tensor([128, 384], F32) as out_t,
        nc.psum_tensor([12, 128], F32) as ps1,
        nc.psum_tensor([128, 384], F32) as ps2,
        nc.semaphore("pk_sem") as pk_sem,
        nc.semaphore("r_sem") as r_sem,
        nc.semaphore("out_sem") as out_sem,
        nc.semaphore("v_sem") as v_sem,
        nc.semaphore("p_sem") as p_sem,
        nc.Block() as block,
    ):
        # Preamble-region DMA triggers: issued before the Block entry
        # barrier so the transfers overlap it.
        nc.sync.dma_start(out=pkt[:], in_=pk[:]).then_inc(pk_sem, 16)
        nc.sync.dma_start(out=rc[:], in_=r12d[:].bitcast(F32R)).then_inc(r_sem, 16)

        @block.sync
        def _(sync: bass.BassEngine):
            sync.wait_ge(v_sem, 15)
            sync.dma_start(out=y[0:64], in_=out_t[0:64, :]).then_inc(out_sem, 16)
            sync.wait_ge(out_sem, 32)

        @block.scalar
        def _(scalar: bass.BassEngine):
            scalar.wait_ge(v_sem, 16)
            scalar.dma_start(out=y[64:128], in_=out_t[64:128, :]).then_inc(out_sem, 16)

        @block.vector
        def _(vector: bass.BassEngine):
            # Every DVE op bumps v_sem on completion; consumers (including
            # same-engine RAW dependents) wait on the running count.
            # Constants first (no input dependency); writes never overlap.
            nc.vector.memset(m128[0:96, 9:12], 0.0).then_inc(v_sem, 1)  # 1
            nc.vector.memset(m128[96:128, 0:9], 0.0).then_inc(v_sem, 1)  # 2
            # rhs1 rows 0..31 = a(p, di=0): zero at p=0
            nc.vector.memset(rhs1[0:32, 0:1], 0.0).then_inc(v_sem, 1)  # 3
            nc.vector.memset(rhs1[0:32, 1:128], 1.0).then_inc(v_sem, 1)  # 4
            # rows 32..63 = a(p, di=1) = 1
            nc.vector.memset(rhs1[32:64, :], 1.0).then_inc(v_sem, 1)  # 5
            # rows 64..95 = a(p, di=2): zero at p=127
            nc.vector.memset(rhs1[64:96, 0:127], 1.0).then_inc(v_sem, 1)  # 6
            nc.vector.memset(rhs1[64:96, 127:128], 0.0).then_inc(v_sem, 1)  # 7
            # rows 96..127 (cneg contraction) = 1
            nc.vector.memset(rhs1[96:128, :], 1.0).then_inc(v_sem, 1)  # 8

            vector.wait_ge(pk_sem, 16)
            W96 = pkt[0:96, 0:9]
            b96 = pkt[0:96, 36:37]
            W32v = pkt[96:128, 9:36].rearrange("ci (co t) -> ci co t", co=3)
            # a1 = W - b ;  |x| = abs_max(x, 0) as a single-immediate op
            nc.vector.tensor_scalar(a1[:], W96, b96, None, ALU.subtract).then_inc(
                v_sem, 1
            )  # 9
            nc.vector.tensor_reduce(
                u96[:],
                W96.rearrange("p (f x) -> p f x", x=1),
                axis=AX.X,
                op=ALU.add,
                apply_absolute_value=True,
            ).then_inc(v_sem, 1)  # 10
            # cneg rows: -sum_t |W|  (independent of t96/u96)
            nc.vector.tensor_reduce(
                m128[96:128, 9:12],
                W32v,
                axis=AX.X,
                op=ALU.add,
                apply_absolute_value=True,
                negate=True,
            ).then_inc(v_sem, 1)  # 11
            vector.wait_ge(v_sem, 9)
            nc.vector.tensor_reduce(
                t96[:],
                a1[:].rearrange("p (f x) -> p f x", x=1),
                axis=AX.X,
                op=ALU.add,
                apply_absolute_value=True,
            ).then_inc(v_sem, 1)  # 12
            vector.wait_ge(v_sem, 12)
            # m128 rows 0..95: |w| - |b-w| per (di,ci)
            nc.vector.tensor_tensor(
                m128[0:96, 0:9], u96[:], t96[:], ALU.subtract
            ).then_inc(v_sem, 1)  # 13

            vector.wait_ge(p_sem, 1)
            nc.vector.tensor_copy(s1[:], ps1[:]).then_inc(v_sem, 1)  # 14
            vector.wait_ge(p_sem, 2)
            nc.vector.tensor_copy(out_t[0:64, :], ps2[0:64, :]).then_inc(v_sem, 1)  # 15
            nc.vector.tensor_copy(out_t[64:128, :], ps2[64:128, :]).then_inc(
                v_sem, 1
            )  # 16

        @block.tensor
        def _(tensor: bass.BassEngine):
            # float32r: single-pass fp32 matmul (vs the LOW/HIGH double pass)
            tensor.wait_ge(v_sem, 13)
            nc.tensor.matmul(ps1[:], m128[:], rhs1[:], start=True, stop=True).then_inc(
                p_sem, 1
            )
            tensor.wait_ge(v_sem, 14)
            tensor.wait_ge(r_sem, 16)
            nc.tensor.matmul(ps2[:], s1[:], rc[:], start=True, stop=True).then_inc(
                p_sem, 1
            )

    return nc


_PROGRAM = None


def _get_program():
    global _PROGRAM
    if _PROGRAM is None:
        _PROGRAM = build_program()
    return _PROGRAM


def kernel(**inputs) -> np.ndarray:
    w26 = np.ascontiguousarray(np.asarray(inputs["w26"], dtype=np.float32))
    b = np.ascontiguousarray(np.asarray(inputs["bn25_b"], dtype=np.float32))
    assert w26.shape == (3, 32, 3, 3) and b.shape == (32,)

    nc = _get_program()
    in_map = {"pk": make_pk(w26, b), "r12const": make_r12()}
    res = run_bass_kernel_spmd(
        nc, [dict(in_map) for _ in range(N_CORES)], list(range(N_CORES))
    )
    # Data-parallel over batch N: core n's output is batch element n.
    return np.stack(
        [
            np.asarray(res.results[n]["y"]).reshape(128, 3, 128).transpose(1, 0, 2)
            for n in range(4)
        ],
        axis=0,
    )


if __name__ == "__main__":
    nc = build_program()
    print("program built OK")


# revision 41
# speedup vs baseline: 1.0468x; 1.0468x over previous
"""Trainium2 Bass kernel for nn_AdderDeconv_new_77034533421672.

Mathematical structure of the reference network:
  - Every adder_l1 layer outputs  -sum |...|  which is strictly negative at
    every position for any generic input.
  - Each adder layer (except the last) is followed by relu(), which therefore
    outputs exactly 0.0 everywhere, and bn_t turns that into the per-channel
    constant map  h[n,c,:,:] = bn*_b[c].
  - MaxUnpool scatters non-positive values into zeros; the following relu
    zeroes those too.
  So the network output equals the last adder layer applied to the constant
  map bn25_b, with zero padding:

    y[n,co,p,q] = -sum_{ci,di,dj} ( inbounds(p+di-1, q+dj-1)
                                      ? |bn25_b[ci] - w26[co,ci,di,dj]|
                                      : |w26[co,ci,di,dj]| )

  This depends only on w26 [3,32,3,3] and bn25_b [32]; it is identical for
  all n.  With a(p,di) = [0 <= p+di-1 < 128], b(q,dj) likewise, and
  wm = |w| - |b-w| (out-of-bounds minus in-bounds tap cost):

    y[co,p,q] = -sum|w26[co,:]| + sum_{di,dj} a(p,di) b(q,dj) wm[co,di,dj]

  Everything after the elementwise |.| is linear, so the device kernel is:
  a couple of DVE ops (subtract + abs-reduce), then TWO matmuls with
  constant 0/1 matrices:
    stage 1 (K=128): [ |w|-|b-w| in (di,ci)-blocks ; -sum_t|w| in ci rows ]
                     against rhs1[., p] = [ a(p,di) ; 1 ]  ->  s1 [12, 128]
    stage 2 (K=12):  s1 against a constant block-diagonal column selector
                     r12 [12, 384]  ->  the full [128, 3*128] map.

  Written in raw Bass (no Tile framework): the dependency graph is a short
  linear chain, explicit semaphores keep every instruction within the HW
  sync-wait slot limits (PE matmul has a single wait slot; raw bass uses
  standalone WAIT instructions instead), and there is no kernel-tail
  drain/barrier overhead.

  Sharding: data-parallel over batch N (hint) — all 8 cores run the identical
  tiny program; the host gathers cores 0..3 as batch elements 0..3.
"""

import numpy as np

import concourse.bass as bass
import concourse.mybir as mybir
from concourse.bass_utils import run_bass_kernel_spmd

F32 = mybir.dt.float32
F32R = mybir.dt.float32r
ALU = mybir.AluOpType
AX = mybir.AxisListType

N_CORES = 8


def make_r12() -> np.ndarray:
    """Constant stage-2 matrix: r12[co*3+dj, co'*128+q] = (co==co')*b(q,dj),
    r12[9+co, co'*128+q] = (co==co')."""
    r12 = np.zeros((12, 384), np.float32)
    for co in range(3):
        for dj in range(3):
            row = np.ones(128, np.float32)
            if dj == 0:
                row[0] = 0.0
            if dj == 2:
                row[127] = 0.0
            r12[co * 3 + dj, co * 128 : (co + 1) * 128] = row
        r12[9 + co, co * 128 : (co + 1) * 128] = 1.0
    return r12


def make_pk(w26: np.ndarray, b: np.ndarray) -> np.ndarray:
    """Host-packed staging tensor (single DMA):
    pk[0:96, 0:9]    = W96[di*32+ci, co*3+dj] = w26[co,ci,di,dj]
    pk[96:128, 9:36] = W32[ci, co*9+t]        = w26[co,ci,t]  (t = di*3+dj)
    pk[0:96, 36]     = b96[di*32+ci]          = bn25_b[ci]
    (W32 lives on partitions 96..127 so its -sum_t|w| reduction lands on the
    same partition lanes as the m128 rows it feeds — DVE is partition-locked.)
    """
    pk = np.zeros((128, 37), np.float32)
    pk[0:96, 0:9] = w26.transpose(2, 1, 0, 3).reshape(96, 9)
    pk[96:128, 9:36] = w26.transpose(1, 0, 2, 3).reshape(32, 27)
    pk[0:96, 36] = np.tile(b, 3)
    return pk


def build_program():
    nc = bass.Bass()
    lp = nc.allow_low_precision(reason="fp32r PE operands; |values| ~ 1e2, threshold 2e-2")
    lp.__enter__()
    pk = nc.dram_tensor("pk", [128, 37], F32, kind="ExternalInput")
    r12d = nc.dram_tensor("r12const", [12, 384], F32, kind="ExternalInput")
    y = nc.dram_tensor("y", [2, 128, 192], F32, kind="ExternalOutput")

    with (
        nc.sbuf_tensor([128, 37], F32) as pkt,
        nc.sbuf_tensor([12, 384], F32R) as rc,
        nc.sbuf_tensor([96, 9], F32) as a1,
        nc.sbuf_tensor([96, 9], F32) as t96,
        nc.sbuf_tensor([96, 9], F32) as u96,
        nc.sbuf_tensor([128, 12], F32) as m128,
        nc.sbuf_tensor([128, 128], F32) as rhs1,
        nc.sbuf_tensor([12, 128], F32R) as s1,
        nc.sbuf_tensor([128, 384], F32) as out_t,
        nc.psum_tensor([128, 512], F32) as ps1f,
        nc.psum_tensor([128, 512], F32) as ps2a,
        nc.psum_tensor([128, 512], F32) as ps2b,
        nc.semaphore("pk_sem") as pk_sem,
        nc.semaphore("r_sem") as r_sem,
        nc.semaphore("out_sem") as out_sem,
        nc.semaphore("v_sem") as v_sem,
        nc.semaphore("p_sem") as p_sem,
    ):
        ps1 = ps1f[0:12, 0:128]

        # True preamble DMA triggers: issued before the Block is even
        # created, so they precede its entry barrier and the transfers
        # overlap all of it.
        nc.sync.dma_start(out=pkt[:], in_=pk[:]).then_inc(pk_sem, 16)
        nc.sync.dma_start(out=rc[:], in_=r12d[:].bitcast(F32R)).then_inc(r_sem, 16)

        blk_ctx = nc.Block()
        block = blk_ctx.__enter__()

        @block.sync
        def _(sync: bass.BassEngine):
            sync.wait_ge(v_sem, 15)
            sync.dma_start(out=y[0], in_=out_t[:, 0:192]).then_inc(out_sem, 16)
            sync.wait_ge(out_sem, 32)

        @block.scalar
        def _(scalar: bass.BassEngine):
            scalar.wait_ge(v_sem, 16)
            scalar.dma_start(out=y[1], in_=out_t[:, 192:384]).then_inc(out_sem, 16)

        @block.vector
        def _(vector: bass.BassEngine):
            # Every DVE op bumps v_sem on completion; consumers (including
            # same-engine RAW dependents) wait on the running count.
            # Constants first (no input dependency); writes never overlap.
            nc.vector.memset(m128[0:96, 9:12], 0.0).then_inc(v_sem, 1)  # 1
            nc.vector.memset(m128[96:128, 0:9], 0.0).then_inc(v_sem, 1)  # 2
            # rhs1 rows 0..31 = a(p, di=0): zero at p=0
            nc.vector.memset(rhs1[0:32, 0:1], 0.0).then_inc(v_sem, 1)  # 3
            nc.vector.memset(rhs1[0:32, 1:128], 1.0).then_inc(v_sem, 1)  # 4
            # rows 32..63 = a(p, di=1) = 1
            nc.vector.memset(rhs1[32:64, :], 1.0).then_inc(v_sem, 1)  # 5
            # rows 64..95 = a(p, di=2): zero at p=127
            nc.vector.memset(rhs1[64:96, 0:127], 1.0).then_inc(v_sem, 1)  # 6
            nc.vector.memset(rhs1[64:96, 127:128], 0.0).then_inc(v_sem, 1)  # 7
            # rows 96..127 (cneg contraction) = 1
            nc.vector.memset(rhs1[96:128, :], 1.0).then_inc(v_sem, 1)  # 8

            vector.wait_ge(pk_sem, 16)
            W96 = pkt[0:96, 0:9]
            b96 = pkt[0:96, 36:37]
            W32v = pkt[96:128, 9:36].rearrange("ci (co t) -> ci co t", co=3)
            # a1 = W - b ;  |x| = abs_max(x, 0) as a single-immediate op
            nc.vector.tensor_scalar(a1[:], W96, b96, None, ALU.subtract).then_inc(
                v_sem, 1
            )  # 9
            nc.vector.tensor_reduce(
                u96[:],
                W96.rearrange("p (f x) -> p f x", x=1),
                axis=AX.X,
                op=ALU.add,
                apply_absolute_value=True,
            ).then_inc(v_sem, 1)  # 10
            # cneg rows: -sum_t |W|  (independent of t96/u96)
            nc.vector.tensor_reduce(
                m128[96:128, 9:12],
                W32v,
                axis=AX.X,
                op=ALU.add,
                apply_absolute_value=True,
                negate=True,
            ).then_inc(v_sem, 1)  # 11
            vector.wait_ge(v_sem, 9)
            nc.vector.tensor_reduce(
                t96[:],
                a1[:].rearrange("p (f x) -> p f x", x=1),
                axis=AX.X,
                op=ALU.add,
                apply_absolute_value=True,
            ).then_inc(v_sem, 1)  # 12
            vector.wait_ge(v_sem, 12)
            # m128 rows 0..95: |w| - |b-w| per (di,ci)
            nc.vector.tensor_tensor(
                m128[0:96, 0:9], u96[:], t96[:], ALU.subtract
            ).then_inc(v_sem, 1)  # 13

            vector.wait_ge(p_sem, 1)
            nc.vector.tensor_copy(s1[:], ps1).then_inc(v_sem, 1)  # 14
            vector.wait_ge(p_sem, 2)
            nc.vector.tensor_copy(out_t[:, 0:192], ps2a[:, 0:192]).then_inc(
                v_sem, 1
            )  # 15
            vector.wait_ge(p_sem, 3)
            nc.vector.tensor_copy(out_t[:, 192:384], ps2b[:, 0:192]).then_inc(
                v_sem, 1
            )  # 16

        @block.tensor
        def _(tensor: bass.BassEngine):
            # float32r: single-pass fp32 matmul (vs the LOW/HIGH double pass)
            tensor.wait_ge(v_sem, 13)
            nc.tensor.matmul(ps1, m128[:], rhs1[:], start=True, stop=True).then_inc(
                p_sem, 1
            )
            tensor.wait_ge(v_sem, 14)
            tensor.wait_ge(r_sem, 16)
            nc.tensor.matmul(
                ps2a[:, 0:192], s1[:], rc[:, 0:192], start=True, stop=True
            ).then_inc(p_sem, 1)
            nc.tensor.matmul(
                ps2b[:, 0:192], s1[:], rc[:, 192:384], start=True, stop=True
            ).then_inc(p_sem, 1)

        blk_ctx.__exit__(None, None, None)

    return nc


_PROGRAM = None


def _get_program():
    global _PROGRAM
    if _PROGRAM is None:
        _PROGRAM = build_program()
    return _PROGRAM


def kernel(**inputs) -> np.ndarray:
    w26 = np.ascontiguousarray(np.asarray(inputs["w26"], dtype=np.float32))
    b = np.ascontiguousarray(np.asarray(inputs["bn25_b"], dtype=np.float32))
    assert w26.shape == (3, 32, 3, 3) and b.shape == (32,)

    nc = _get_program()
    in_map = {"pk": make_pk(w26, b), "r12const": make_r12()}
    res = run_bass_kernel_spmd(
        nc, [dict(in_map) for _ in range(N_CORES)], list(range(N_CORES))
    )
    # Data-parallel over batch N: core n's output is batch element n.
    return np.stack(
        [
            np.concatenate(list(np.asarray(res.results[n]["y"])), axis=1)
            .reshape(128, 3, 128)
            .transpose(1, 0, 2)
            for n in range(4)
        ],
        axis=0,
    )


if __name__ == "__main__":
    nc = build_program()
    print("program built OK")
